# revision 1
# baseline (speedup 1.0000x reference)
"""Cross-attention Trainium2 kernel (Bass/Tile), SPMD over 8 NeuronCores.

Problem: b=8, i=j=2048, query/context dim 512, inner dim 256.
Sharding: data-parallel over batch — one batch element per core, no
collectives. Each core computes, for its batch element:

    q = x @ Wq ; k = ctx @ Wk ; v = ctx @ Wv
    sim = (q @ k^T) * d^-0.5 ; attn = softmax_j(sim) masked on j
    out = attn @ v ; y = out @ Wo + bo + x

Per-core dataflow (all matmuls bf16 with fp32 PSUM accumulation):
  1. Cast x and ctx to bf16, then PE-transpose (1 cyc/row) into
     k-major layout (xT, ctxT).
  2. qT = Wq^T @ xT, kT = Wk^T @ ctxT (d-major); v = ctxT^T @ Wv
     (j-major natural). The mask is folded into the v copy as a
     per-partition (per-j) scale, zeroing rows of v for masked j.
  3. Per 512-col i-block: simT[j, i] = kT^T @ qT -> PSUM, computed
     TRANSPOSED so the exp on ScalarE (values are bounded, so no max
     subtraction is needed) writes the j-major layout the PV matmul
     needs directly — no 128x128 attn transposes or extra copies.
  4. outT = v^T @ attnT accumulated over all j; softmax denominators
     come from a mask-weighted ones-row matmul on PE
     (denom = mask^T @ exp, which is also the correctly-masked sum),
     transposed to i-major via tiny K=1 matmuls.
  5. y = outT^T @ Wo, scaled by the softmax reciprocal per row (row
     scaling commutes with the right-multiply by Wo), plus x and bo.
"""

import sys

import numpy as np

if "/opt/trn_rl_repo" not in sys.path:
    sys.path.insert(0, "/opt/trn_rl_repo")

_P = 128          # partitions
_B = 8            # batch == number of cores
_I = 2048         # query sequence length
_J = 2048         # context sequence length
_K = 512          # query/context feature dim
_D = 256          # inner dim
_NB = 512         # matmul free-dim block
_KT = _K // _P    # 4 contraction tiles for projections
_DT = _D // _P    # 2 inner-dim tiles
_IT = _I // _P    # 16 i tiles
_JT = _J // _P    # 16 j tiles
_SCALE = float(_D) ** -0.5

_CACHE = {}


def _split_multi_waits(nc, limits):
    """Walrus in this container rejects instructions carrying more sem
    waits than its per-template slot count (e.g. Drain allows 1). Move
    excess waits onto wait-only Drain carriers on the same engine,
    inserted just before the instruction — semantically identical."""
    from concourse import mybir

    n_split = 0
    for func in nc.m.functions:
        for block in func.blocks:
            out = []
            for inst in block.instructions:
                si = inst.sync_info
                maxw = limits.get(type(inst).__name__, limits.get("*"))
                if (
                    maxw is not None
                    and si is not None
                    and si.on_wait
                    and len(si.on_wait) > maxw
                ):
                    waits = list(si.on_wait)
                    keep, rest = waits[:maxw], waits[maxw:]
                    for i in range(0, len(rest), 1):
                        car = mybir.InstDrain(
                            name=f"I-waitcar-{nc.next_id()}", ins=[], outs=[]
                        )
                        car.engine = inst.engine
                        car.sync_info = mybir.SyncInfo(
                            on_wait=[rest[i]], on_update=[]
                        )
                        nc.register_instruction(car)
                        out.append(car)
                        n_split += 1
                    inst.sync_info = mybir.SyncInfo(
                        on_wait=keep, on_update=list(si.on_update or [])
                    )
                out.append(inst)
            block.instructions = out
    return n_split


def _build_nc(repeat=1):
    import concourse.bass as bass
    import concourse.tile as tile
    from concourse import mybir
    from concourse.masks import make_identity

    dt = mybir.dt
    Alu = mybir.AluOpType
    Act = mybir.ActivationFunctionType

    nc = bass.Bass("TRN2", target_bir_lowering=False)

    x_d = nc.dram_tensor("x", [_I, _K], dt.float32, kind="ExternalInput")
    c_d = nc.dram_tensor("context", [_J, _K], dt.float32, kind="ExternalInput")
    m_d = nc.dram_tensor("mask", [_J], dt.uint8, kind="ExternalInput")
    wq_d = nc.dram_tensor("Wq", [_K, _D], dt.float32, kind="ExternalInput")
    wk_d = nc.dram_tensor("Wk", [_K, _D], dt.float32, kind="ExternalInput")
    wv_d = nc.dram_tensor("Wv", [_K, _D], dt.float32, kind="ExternalInput")
    wo_d = nc.dram_tensor("Wo", [_D, _K], dt.float32, kind="ExternalInput")
    bo_d = nc.dram_tensor("bo", [_K], dt.float32, kind="ExternalInput")
    y_d = nc.dram_tensor("out", [_I, _K], dt.float32, kind="ExternalOutput")

    with tile.TileContext(nc) as tc:
        with (
            tc.tile_pool(name="persist", bufs=1) as persist,
            tc.tile_pool(name="stage", bufs=3) as stage,
            tc.tile_pool(name="small", bufs=4) as small,
            tc.tile_pool(name="attnT", bufs=2) as attntp,
            tc.tile_pool(name="yout", bufs=3) as youtp,
            tc.tile_pool(name="psmm", bufs=2, space="PSUM") as psmm,
            tc.tile_pool(name="psacc", bufs=2, space="PSUM") as psacc,
            tc.tile_pool(name="pstr", bufs=3, space="PSUM") as pstr,
        ):
            # ---------------- constants / weights ----------------
            ident_b = persist.tile([_P, _P], dt.bfloat16, tag="identb")
            make_identity(nc, ident_b)
            one_one = persist.tile([1, 1], dt.float32, tag="one_one")
            nc.vector.memset(one_one, 1.0)

            # ---------------- persistent activations ----------------
            x_nat = persist.tile([_P, _IT, _K], dt.float32, tag="xnat")
            xT = persist.tile([_P, _KT, _I], dt.bfloat16, tag="xT")
            cT = persist.tile([_P, _KT, _J], dt.bfloat16, tag="cT")
            qT = persist.tile([_P, _DT, _I], dt.bfloat16, tag="qT")
            kT = persist.tile([_P, _DT, _J], dt.bfloat16, tag="kT")
            v = persist.tile([_P, _JT, _D], dt.bfloat16, tag="v")
            oT = persist.tile([_P, _DT, _I], dt.bfloat16, tag="oT")
            recips = persist.tile([_P, _IT], dt.float32, tag="recips")

            wq = persist.tile([_P, _KT, _D], dt.bfloat16, tag="wq")
            wk = persist.tile([_P, _KT, _D], dt.bfloat16, tag="wk")
            wv = persist.tile([_P, _KT, _D], dt.bfloat16, tag="wv")
            wo = persist.tile([_P, _DT, _K], dt.bfloat16, tag="wo")
            mask01 = persist.tile([_P, _JT], dt.float32, tag="mask01")
            mask01b = persist.tile([_P, _JT], dt.bfloat16, tag="mask01b")
            bo_bc = persist.tile([_P, _K], dt.float32, tag="bobc")

            # `repeat` > 1 chains extra full iterations for timing
            # calibration; WAW deps on the persistent tiles serialize
            # them so (t_N - t_1)/(N-1) approximates one iteration.
            for _rep in range(repeat):
                # ---------------- input loading + transposes ----------------
                # DMA issue order matters: SP's HWDGE is a FIFO. Load the
                # first ctx tiles before the weights so the PE starts
                # transposing immediately; x rides behind.
                def load_weights_early():
                    for w_dram, w_sb, nt in ((wk_d, wk, _KT), (wv_d, wv, _KT)):
                        ws = stage.tile([_P, nt, _D], dt.float32, tag="wstage",
                                        name=f"ws_{w_sb.name}")
                        nc.sync.dma_start(
                            out=ws, in_=w_dram[:].rearrange("(t p) d -> p t d", p=_P)
                        )
                        nc.vector.tensor_copy(out=w_sb, in_=ws)
                    msk8 = small.tile([_P, _JT], dt.uint8, tag="msk8")
                    nc.sync.dma_start(
                        out=msk8, in_=m_d[:].rearrange("(t p) -> p t", p=_P)
                    )
                    nc.vector.tensor_copy(out=mask01, in_=msk8)
                    nc.vector.tensor_copy(out=mask01b, in_=mask01)

                def load_weights_late():
                    for w_dram, w_sb in ((wq_d, wq),):
                        ws = stage.tile([_P, _KT, _D], dt.float32, tag="wstage",
                                        name=f"ws_{w_sb.name}")
                        nc.sync.dma_start(
                            out=ws, in_=w_dram[:].rearrange("(t p) d -> p t d", p=_P)
                        )
                        nc.vector.tensor_copy(out=w_sb, in_=ws)
                    ws = stage.tile([_P, _DT, _K], dt.float32, tag="wstage",
                                    name="ws_wo")
                    nc.sync.dma_start(
                        out=ws, in_=wo_d[:].rearrange("(t p) k -> p t k", p=_P)
                    )
                    nc.vector.tensor_copy(out=wo, in_=ws)
                    bo_ap = bo_d[:]
                    nc.sync.dma_start(
                        out=bo_bc,
                        in_=bass.AP(
                            tensor=bo_ap.tensor, offset=bo_ap.offset,
                            ap=[[0, _P], bo_ap.ap[0]],
                        ),
                    )

                # ctx -> ctxT (bf16, k on partitions) via PE transposes.
                # 1MiB DMAs (4 row-tiles each) keep the feed rate above
                # the PE's transpose+projection consumption rate.
                for g in range(_JT // 4):
                    cn = stage.tile([_P, 4, _K], dt.float32, tag="cnat",
                                    bufs=3)
                    # Half-group (512KB) DMAs: the casts/transposes for
                    # the first two tiles start as soon as the first half
                    # lands. The very first tile gets its own 256KB DMA
                    # so the PE's first transpose starts earliest.
                    if g == 0:
                        splits = ((0, 1), (1, 1), (2, 2))
                    else:
                        splits = ((0, 2), (2, 2))
                    for o, n in splits:
                        r0 = (g * 4 + o) * _P
                        nc.sync.dma_start(
                            out=cn[:, o:o + n, :],
                            in_=c_d[r0:r0 + n * _P, :].rearrange(
                                "(t p) k -> p t k", p=_P
                            ),
                        )
                    if g == 1:
                        load_weights_early()
                    for tt in range(4):
                        jt = g * 4 + tt
                        # bf16 transposes run at 1 cyc/row (vs 2 for f32);
                        # the pre-cast rides on ScalarE.
                        cnb = stage.tile([_P, _K], dt.bfloat16, tag="cnb",
                                         bufs=4)
                        nc.vector.tensor_copy(out=cnb, in_=cn[:, tt, :])
                        for kt in range(_KT):
                            tr = pstr.tile([_P, _P], dt.bfloat16, tag="tr")
                            nc.tensor.transpose(
                                tr, cnb[:, kt * _P:(kt + 1) * _P], ident_b
                            )
                            dst = cT[:, kt, jt * _P:(jt + 1) * _P]
                            if kt % 2 == 0:
                                # DVE moves bits; uint32 view halves the
                                # element count (bf16 pair per lane).
                                nc.vector.tensor_copy(
                                    out=dst.bitcast(dt.uint32),
                                    in_=tr[:].bitcast(dt.uint32),
                                )
                            else:
                                nc.scalar.copy(out=dst, in_=tr)

                # kT projection: kT[d, j] = Wk^T @ ctxT
                for dh in range(_DT):
                    for jb in range(_J // _NB):
                        ps = psmm.tile([_P, _NB], dt.float32, tag="mm")
                        for kt in range(_KT):
                            nc.tensor.matmul(
                                ps,
                                lhsT=wk[:, kt, dh * _P:(dh + 1) * _P],
                                rhs=cT[:, kt, jb * _NB:(jb + 1) * _NB],
                                start=(kt == 0), stop=(kt == _KT - 1),
                            )
                        nc.vector.tensor_copy(
                            out=kT[:, dh, jb * _NB:(jb + 1) * _NB], in_=ps
                        )

                # v projection: v[j, d] = ctxT^T @ Wv, mask folded in as a
                # per-j scale on the PSUM->SBUF copy.
                for jt in range(_JT):
                    ps = psmm.tile([_P, _D], dt.float32, tag="mm")
                    for kt in range(_KT):
                        nc.tensor.matmul(
                            ps,
                            lhsT=cT[:, kt, jt * _P:(jt + 1) * _P],
                            rhs=wv[:, kt, :],
                            start=(kt == 0), stop=(kt == _KT - 1),
                        )
                    nc.scalar.activation(
                        out=v[:, jt, :], in_=ps, func=Act.Copy,
                        scale=mask01[:, jt:jt + 1],
                    )

                # x -> x_nat (kept for the residual) and xT
                for g in range(_IT // 4):
                    for h in range(2):
                        r0 = (g * 4 + h * 2) * _P
                        nc.sync.dma_start(
                            out=x_nat[:, g * 4 + h * 2:g * 4 + (h + 1) * 2, :],
                            in_=x_d[r0:r0 + 2 * _P, :].rearrange(
                                "(t p) k -> p t k", p=_P
                            ),
                        )
                    if g == 0:
                        load_weights_late()
                    for tt in range(4):
                        it = g * 4 + tt
                        xbf = stage.tile([_P, _K], dt.bfloat16, tag="xbf",
                                         bufs=4)
                        nc.vector.tensor_copy(out=xbf, in_=x_nat[:, it, :])
                        for kt in range(_KT):
                            tr = pstr.tile([_P, _P], dt.bfloat16, tag="tr")
                            nc.tensor.transpose(
                                tr, xbf[:, kt * _P:(kt + 1) * _P], ident_b
                            )
                            dst = xT[:, kt, it * _P:(it + 1) * _P]
                            if kt % 2 == 0:
                                nc.vector.tensor_copy(
                                    out=dst.bitcast(dt.uint32),
                                    in_=tr[:].bitcast(dt.uint32),
                                )
                            else:
                                nc.scalar.copy(out=dst, in_=tr)
                        # bo rides in the residual; GpSimd is otherwise
                        # idle, and this is off the critical path.
                        nc.gpsimd.tensor_add(
                            out=x_nat[:, it, :], in0=x_nat[:, it, :],
                            in1=bo_bc
                        )

                # qT projection: qT[d, i] = Wq^T @ xT
                for dh in range(_DT):
                    for ib in range(_I // _NB):
                        ps = psmm.tile([_P, _NB], dt.float32, tag="mm")
                        for kt in range(_KT):
                            nc.tensor.matmul(
                                ps,
                                lhsT=wq[:, kt, dh * _P:(dh + 1) * _P],
                                rhs=xT[:, kt, ib * _NB:(ib + 1) * _NB],
                                start=(kt == 0), stop=(kt == _KT - 1),
                            )
                        nc.vector.tensor_copy(
                            out=qT[:, dh, ib * _NB:(ib + 1) * _NB], in_=ps
                        )

                # ---------------- attention main loop ----------------
                # Software pipeline at j-tile granularity: while block b's
                # simT+exp stream through PSUM, the PE interleaves block
                # b-1's PV and denominator matmuls — ScalarE's exp
                # (~612ns/tile) is slower than the sim pair (~426ns), so
                # without the interleave the PE stalls on PSUM recycling.
                aTs = {}
                accs = {}
                pdens = {}

                def start_block(b):
                    aTs[b] = attntp.tile(
                        [_P, _JT, _NB], dt.bfloat16, tag="aT", name=f"aT{b}"
                    )
                    accs[b] = [
                        psacc.tile([_P, _NB], dt.float32, tag="acc",
                                   name=f"acc{b}_{dh}")
                        for dh in range(_DT)
                    ]
                    pdens[b] = pstr.tile([1, _NB], dt.float32, tag="den",
                                         bufs=1, name=f"pden{b}")

                def sim_exp(b, jt):
                    ps = psmm.tile([_P, _NB], dt.float32, tag="mm")
                    for dh in range(_DT):
                        nc.tensor.matmul(
                            ps,
                            lhsT=kT[:, dh, jt * _P:(jt + 1) * _P],
                            rhs=qT[:, dh, b * _NB:(b + 1) * _NB],
                            start=(dh == 0), stop=(dh == _DT - 1),
                        )
                    nc.scalar.activation(
                        out=aTs[b][:, jt, :], in_=ps,
                        func=Act.Exp, bias=0.0, scale=_SCALE,
                    )

                def pv_denom(b, jt):
                    aT = aTs[b]
                    for dh in range(_DT):
                        nc.tensor.matmul(
                            accs[b][dh],
                            lhsT=v[:, jt, dh * _P:(dh + 1) * _P],
                            rhs=aT[:, jt, :],
                            start=(jt == 0), stop=(jt == _JT - 1),
                        )
                    nc.tensor.matmul(
                        pdens[b],
                        lhsT=mask01b[:, jt:jt + 1],
                        rhs=aT[:, jt, :],
                        start=(jt == 0), stop=(jt == _JT - 1),
                    )

                def finish_block(b):
                    for dh in range(_DT):
                        nc.vector.tensor_copy(
                            out=oT[:, dh, b * _NB:(b + 1) * _NB],
                            in_=accs[b][dh],
                        )
                    del accs[b], aTs[b]
                    den_sb = small.tile([1, _NB], dt.float32, tag="densb")
                    nc.vector.tensor_copy(out=den_sb, in_=pdens.pop(b))
                    # Transpose denom to i-major via K=1 matmuls, then 1/x.
                    for tt in range(4):
                        t = b * 4 + tt
                        trd = pstr.tile([_P, 1], dt.float32, tag="tr",
                                        name=f"trd{t}")
                        nc.tensor.matmul(
                            trd,
                            lhsT=den_sb[:, tt * _P:(tt + 1) * _P],
                            rhs=one_one,
                            start=True, stop=True,
                        )
                        nc.vector.reciprocal(out=recips[:, t:t + 1], in_=trd)
                    # Output projection + softmax normalization + residual.
                    for tt in range(4):
                        t = b * 4 + tt
                        yp = psmm.tile([_P, _K], dt.float32, tag="mm")
                        for dh in range(_DT):
                            nc.tensor.matmul(
                                yp,
                                lhsT=oT[:, dh, t * _P:(t + 1) * _P],
                                rhs=wo[:, dh, :],
                                start=(dh == 0), stop=(dh == _DT - 1),
                            )
                        ys = youtp.tile([_P, _K], dt.float32, tag="ys")
                        nc.vector.scalar_tensor_tensor(
                            out=ys, in0=yp, scalar=recips[:, t:t + 1],
                            in1=x_nat[:, t, :], op0=Alu.mult, op1=Alu.add,
                        )
                        nc.sync.dma_start(
                            out=y_d[t * _P:(t + 1) * _P, :], in_=ys
                        )

                n_blocks = _I // _NB
                for b in range(n_blocks):
                    start_block(b)
                    for jt in range(_JT):
                        sim_exp(b, jt)
                        if b > 0:
                            pv_denom(b - 1, jt)
                    if b > 0:
                        finish_block(b - 1)
                for jt in range(_JT):
                    pv_denom(n_blocks - 1, jt)
                finish_block(n_blocks - 1)

    _split_multi_waits(nc, {"*": 1})
    nc.finalize()
    return nc


def kernel(x, context, mask, Wq, Wk, Wv, Wo, bo):
    from concourse.bass_utils import run_bass_kernel_spmd

    if "nc" not in _CACHE:
        _CACHE["nc"] = _build_nc()
    nc = _CACHE["nc"]

    x = np.ascontiguousarray(np.asarray(x, dtype=np.float32))
    context = np.ascontiguousarray(np.asarray(context, dtype=np.float32))
    mask_u8 = np.ascontiguousarray(np.asarray(mask).astype(np.uint8))
    shared = {
        "Wq": np.ascontiguousarray(np.asarray(Wq, dtype=np.float32)),
        "Wk": np.ascontiguousarray(np.asarray(Wk, dtype=np.float32)),
        "Wv": np.ascontiguousarray(np.asarray(Wv, dtype=np.float32)),
        "Wo": np.ascontiguousarray(np.asarray(Wo, dtype=np.float32)),
        "bo": np.ascontiguousarray(np.asarray(bo, dtype=np.float32)),
    }
    in_maps = [
        {"x": x[b], "context": context[b], "mask": mask_u8[b], **shared}
        for b in range(_B)
    ]
    res = run_bass_kernel_spmd(nc, in_maps, core_ids=list(range(_B)))
    return np.stack([res.results[b]["out"] for b in range(_B)], axis=0)

